# revision 1
# baseline (speedup 1.0000x reference)
"""Llama decode block (single token) on 8 TRN2 NeuronCores, tensor-parallel.

Sharding (per sharding_hint): w_q/w_k/w_v/w_ff1 column-sharded, w_o/w_ff2
row-sharded, KV cache sharded by head (4 heads/core). AllReduce after the
attention output projection and after w_ff2. The residual stream x is folded
into the all-reduces as x/8 per core, so each AR output is the full residual
sum directly.

Per-core dataflow (all matvecs run on the PE with the activation vector as
the stationary operand, streaming the weights as the moving operand):
  rmsnorm(x) -> h_cols[128,32]  (PE transpose of [32,128] rows)
  q/k/v[1,512] = h @ W         (32 k-blocks accumulated in PSUM)
  RoPE on q,k rows; q pre-scaled by 1/sqrt(128)
  scores: per 128-token tile, fused DVE multiply+reduce against K tiles
  softmax without max-subtraction (scores are O(8), exp is safe in f32)
  attn@V: per-tile PE matmuls, V tile stationary -> o[128(d),4(h)] cols
  o @ w_o + x/8 -> [1,4096] -> AllReduce #1 -> x2
  rmsnorm(x2) -> h2_cols; silu(h2 @ w_ff1) -> a[128,11] cols
  a @ w_ff2 + x2/8 -> [1,4096] -> AllReduce #2 -> output
"""

import math

import numpy as np

import concourse.bass as bass
import concourse.mybir as mybir
import concourse.tile as tile
from concourse import bacc
from concourse import bass_utils

F32 = mybir.dt.float32
AF = mybir.ActivationFunctionType
ALU = mybir.AluOpType

HIDDEN = 4096
N_HEADS = 32
HEAD_DIM = 128
INTERM = 11008
KV_LEN = 4096
N_CORES = 8

HEADS_PC = N_HEADS // N_CORES          # 4 heads per core
QKV_N = HEADS_PC * HEAD_DIM            # 512
FF_N = INTERM // N_CORES               # 1376
KB = HIDDEN // 128                     # 32 k-blocks of the hidden dim
T_TILES = KV_LEN // 128                # 32 token tiles
FF_KB_SIZES = [128] * 10 + [96]        # 1376 = 10*128 + 96
SCALE = 1.0 / math.sqrt(HEAD_DIM)


def _emit(nc, tc):
    i = {}  # dram input handles

    def din(name, shape):
        i[name] = nc.dram_tensor(name, list(shape), F32, kind="ExternalInput").ap()

    din("x", [HIDDEN])
    din("attn_norm", [HIDDEN])
    din("ffn_norm", [HIDDEN])
    din("sin", [HEAD_DIM // 2])
    din("ident32", [32, 32])
    din("cos", [HEAD_DIM // 2])
    din("wq", [HIDDEN, QKV_N])
    din("wk", [HIDDEN, QKV_N])
    din("wv", [HIDDEN, QKV_N])
    din("wo", [QKV_N, HIDDEN])
    din("kc", [KV_LEN, QKV_N])
    din("vc", [KV_LEN, QKV_N])
    din("wf1", [HIDDEN, FF_N])
    din("wf2", [FF_N, HIDDEN])
    y = nc.dram_tensor("y", [HIDDEN], F32, kind="ExternalOutput").ap()

    with (
        tc.tile_pool(name="const", bufs=1) as cpool,
        tc.tile_pool(name="wbig", bufs=4) as wpool,
        tc.tile_pool(name="kpool", bufs=2) as kpool,
        tc.tile_pool(name="vpool", bufs=2) as vpool,
        tc.tile_pool(name="sm", bufs=1) as sm,
        tc.tile_pool(name="scr", bufs=4) as scr,
        tc.tile_pool(name="psum", bufs=8, space="PSUM") as pp,
        tc.tile_pool(name="dram", bufs=1, space="DRAM") as dram,
    ):
        # ---- constants ----
        ones32 = cpool.tile([32, 1], F32)
        nc.vector.memset(ones32[:], 1.0)
        ones128 = cpool.tile([128, 1], F32)
        nc.vector.memset(ones128[:], 1.0)
        eighth = cpool.tile([1, 1], F32)
        nc.vector.memset(eighth[:], 1.0 / N_CORES)
        eps11 = cpool.tile([1, 1], F32)
        nc.vector.memset(eps11[:], 1e-6)
        ident32 = cpool.tile([32, 32], F32)
        nc.sync.dma_start(ident32[:], i["ident32"])
        ones_r32 = cpool.tile([1, 32], F32)
        nc.vector.memset(ones_r32[:], 1.0)
        ones_r128 = cpool.tile([1, 128], F32)
        nc.vector.memset(ones_r128[:], 1.0)

        sin_sb = cpool.tile([1, 64], F32)
        cos_sb = cpool.tile([1, 64], F32)
        nc.sync.dma_start(sin_sb[:], i["sin"].rearrange("(a d) -> a d", a=1))
        nc.sync.dma_start(cos_sb[:], i["cos"].rearrange("(a d) -> a d", a=1))
        sinq = cpool.tile([1, 64], F32)
        cosq = cpool.tile([1, 64], F32)
        nc.vector.tensor_scalar_mul(sinq[:], sin_sb[:], SCALE)
        nc.vector.tensor_scalar_mul(cosq[:], cos_sb[:], SCALE)

        # ---- rmsnorm #1 -> h_cols [128, 32] ----
        def rmsnorm_cols(x_dram, norm_dram, tag):
            x_rows = sm.tile([32, 128], F32, name=f"x_rows_{tag}", tag="x_rows")
            nrm_rows = sm.tile([32, 128], F32, name=f"nrm_rows_{tag}", tag="nrm_rows")
            nc.sync.dma_start(x_rows[:], x_dram.rearrange("(a d) -> a d", a=32))
            nc.sync.dma_start(nrm_rows[:], norm_dram.rearrange("(a d) -> a d", a=32))
            sq = sm.tile([32, 128], F32, name=f"sq_{tag}", tag="sq")
            ssq = sm.tile([32, 1], F32, name=f"ssq_{tag}", tag="ssq")
            nc.scalar.activation(sq[:], x_rows[:], AF.Square, accum_out=ssq[:])
            ms_psum = pp.tile([1, 1], F32, name=f"ms_psum_{tag}", tag="ps")
            nc.tensor.matmul(ms_psum[:], ones32[:], ssq[:])
            rstd = sm.tile([1, 1], F32, name=f"rstd_{tag}", tag="rstd")
            # sqrt(mean + eps), then reciprocal (Rsqrt activation is banned)
            nc.scalar.activation(rstd[:], ms_psum[:], AF.Sqrt,
                                 bias=eps11[:], scale=1.0 / HIDDEN)
            nc.vector.reciprocal(rstd[:], rstd[:])
            rstd_ps = pp.tile([32, 1], F32, name=f"rstd_ps_{tag}", tag="ps")
            nc.tensor.matmul(rstd_ps[:], ones_r32[:], rstd[:])
            rstd32 = sm.tile([32, 1], F32, name=f"rstd32_{tag}", tag="rstd32")
            nc.vector.tensor_copy(rstd32[:], rstd_ps[:])
            h_rows = sm.tile([32, 128], F32, name=f"h_rows_{tag}", tag="h_rows")
            nc.vector.tensor_tensor(h_rows[:], x_rows[:], nrm_rows[:], ALU.mult)
            nc.vector.tensor_scalar_mul(h_rows[:], h_rows[:], rstd32[:])
            h_psum = pp.tile([128, 32], F32, name=f"h_psum_{tag}", tag="ps")
            nc.tensor.transpose(h_psum[:], h_rows[:], ident32[:])
            h_cols = sm.tile([128, 32], F32, name=f"h_cols_{tag}", tag="hcols")
            nc.vector.tensor_copy(h_cols[:], h_psum[:])
            return h_cols

        h_cols = rmsnorm_cols(i["x"], i["attn_norm"], "a")

        # ---- q/k/v = h @ W (h stationary, weights moving) ----
        qkv_rows = {}
        for wname in ("wq", "wk", "wv"):
            ps = pp.tile([1, QKV_N], F32, name=f"ps_{wname}", tag="ps")
            for t in range(4):
                wt = wpool.tile([128, 8, 512], F32, name=f"{wname}_t", tag="w")
                nc.sync.dma_start(
                    wt[:],
                    i[wname][t * 1024:(t + 1) * 1024, :].rearrange(
                        "(b p) c -> p b c", p=128),
                )
                for b in range(8):
                    kb = t * 8 + b
                    nc.tensor.matmul(
                        ps[:], h_cols[:, kb:kb + 1], wt[:, b, :],
                        start=(kb == 0), stop=(kb == KB - 1),
                    )
            row = sm.tile([1, QKV_N], F32, name=f"{wname}_row")
            nc.scalar.copy(row[:], ps[:])
            qkv_rows[wname] = row

        # ---- RoPE on q (pre-scaled by 1/sqrt(d)) and k ----
        def rope(row, cos_t, sin_t, tag):
            out = sm.tile([1, QKV_N], F32, name=f"rope_{tag}")
            tmp = sm.tile([1, QKV_N], F32, name=f"rope_tmp_{tag}")
            r3 = row[:].rearrange("a (h d) -> a h d", h=HEADS_PC)
            o3 = out[:].rearrange("a (h d) -> a h d", h=HEADS_PC)
            t3 = tmp[:].rearrange("a (h d) -> a h d", h=HEADS_PC)
            x1, x2 = r3[:, :, 0:64], r3[:, :, 64:128]
            cb = cos_t[:].unsqueeze(1).to_broadcast((1, HEADS_PC, 64))
            sb = sin_t[:].unsqueeze(1).to_broadcast((1, HEADS_PC, 64))
            nc.vector.tensor_tensor(o3[:, :, 0:64], x1, cb, ALU.mult)
            nc.vector.tensor_tensor(t3[:, :, 0:64], x2, sb, ALU.mult)
            nc.vector.tensor_sub(o3[:, :, 0:64], o3[:, :, 0:64], t3[:, :, 0:64])
            nc.vector.tensor_tensor(o3[:, :, 64:128], x2, cb, ALU.mult)
            nc.vector.tensor_tensor(t3[:, :, 64:128], x1, sb, ALU.mult)
            nc.vector.tensor_add(o3[:, :, 64:128], o3[:, :, 64:128],
                                 t3[:, :, 64:128])
            return out

        q_rot = rope(qkv_rows["wq"], cosq, sinq, "q")
        k_rot = rope(qkv_rows["wk"], cos_sb, sin_sb, "k")
        v_row = qkv_rows["wv"]

        q_rep = sm.tile([128, QKV_N], F32, name="q_rep")
        qrep_ps = pp.tile([128, QKV_N], F32, name="qrep_ps", tag="ps")
        nc.tensor.matmul(qrep_ps[:], ones_r128[:], q_rot[:])
        nc.vector.tensor_copy(q_rep[:], qrep_ps[:])

        # ---- attention over the KV cache ----
        o_psum = pp.tile([128, HEADS_PC], F32, name="o_psum", tag="ps")
        denom_acc = sm.tile([128, HEADS_PC], F32, name="denom_acc")
        nc.vector.memset(denom_acc[:], 0.0)

        for st in range(4):
            k_sup = kpool.tile([128, 8, 512], F32, name="k_sup", tag="k")
            v_sup = vpool.tile([128, 8, 512], F32, name="v_sup", tag="v")
            nc.sync.dma_start(
                k_sup[:],
                i["kc"][st * 1024:(st + 1) * 1024, :].rearrange(
                    "(b p) c -> p b c", p=128),
            )
            nc.sync.dma_start(
                v_sup[:],
                i["vc"][st * 1024:(st + 1) * 1024, :].rearrange(
                    "(b p) c -> p b c", p=128),
            )
            for b in range(8):
                gt = st * 8 + b
                scores = scr.tile([128, HEADS_PC], F32, name="scores", tag="sc")
                scratch = scr.tile([128, QKV_N], F32, name="scratch", tag="scratch")
                nc.vector.tensor_tensor(scratch[:], k_sup[:, b, :], q_rep[:],
                                        ALU.mult)
                nc.vector.tensor_reduce(
                    scores[:],
                    scratch[:].rearrange("p (h d) -> p h d", h=HEADS_PC),
                    mybir.AxisListType.X, ALU.add)
                expt = scr.tile([128, HEADS_PC], F32, name="expt", tag="expt")
                nc.scalar.activation(expt[:], scores[:], AF.Exp)
                nc.vector.tensor_add(denom_acc[:], denom_acc[:], expt[:])
                for h in range(HEADS_PC):
                    # start clears has_written for the whole PSUM bank, so
                    # only the very first matmul into o_psum may set it.
                    nc.tensor.matmul(
                        o_psum[:, h:h + 1],
                        v_sup[:, b, h * 128:(h + 1) * 128],
                        expt[:, h:h + 1],
                        start=(gt == 0 and h == 0), stop=False,
                    )

        # current-token contribution (position KV_LEN)
        s_new = sm.tile([1, HEADS_PC], F32, name="s_new")
        scr_new = sm.tile([1, QKV_N], F32, name="scr_new")
        nc.vector.tensor_tensor(scr_new[:], q_rot[:], k_rot[:], ALU.mult)
        nc.vector.tensor_reduce(
            s_new[:],
            scr_new[:].rearrange("a (h d) -> a h d", h=HEADS_PC),
            mybir.AxisListType.X, ALU.add)
        e_new = sm.tile([1, HEADS_PC], F32, name="e_new")
        nc.scalar.activation(e_new[:], s_new[:], AF.Exp)
        for h in range(HEADS_PC):
            nc.tensor.matmul(
                o_psum[:, h:h + 1],
                v_row[:, h * 128:(h + 1) * 128],
                e_new[:, h:h + 1],
                start=False, stop=(h == HEADS_PC - 1),
            )

        denom_psum = pp.tile([1, HEADS_PC], F32, name="denom_psum", tag="ps")
        nc.tensor.matmul(denom_psum[:], ones128[:], denom_acc[:])
        denom = sm.tile([1, HEADS_PC], F32, name="denom")
        nc.vector.tensor_copy(denom[:], denom_psum[:])
        nc.vector.tensor_add(denom[:], denom[:], e_new[:])
        nc.vector.reciprocal(denom[:], denom[:])
        recip_ps = pp.tile([128, HEADS_PC], F32, name="recip_ps", tag="ps")
        nc.tensor.matmul(recip_ps[:], ones_r128[:], denom[:])
        recip_bc = sm.tile([128, HEADS_PC], F32, name="recip_bc")
        nc.vector.tensor_copy(recip_bc[:], recip_ps[:])
        o_sb = sm.tile([128, HEADS_PC], F32, name="o_sb")
        nc.vector.tensor_tensor(o_sb[:], o_psum[:], recip_bc[:], ALU.mult)

        # ---- o @ w_o + x/8 -> [1,4096] -> AllReduce #1 ----
        x_row = sm.tile([1, HIDDEN], F32, name="x_row", tag="xrow")
        nc.sync.dma_start(x_row[:], i["x"].rearrange("(a d) -> a d", a=1))

        chunks1 = [pp.tile([1, 512], F32, name=f"c1_{n}", tag="ps")
                   for n in range(8)]
        for kb in range(HEADS_PC):
            wo_t = wpool.tile([128, HIDDEN], F32, name="wo_t", tag="w")
            nc.sync.dma_start(wo_t[:], i["wo"][kb * 128:(kb + 1) * 128, :])
            for n in range(8):
                nc.tensor.matmul(
                    chunks1[n][:], o_sb[:, kb:kb + 1],
                    wo_t[:, n * 512:(n + 1) * 512],
                    start=(kb == 0), stop=False,
                )
        o_row = sm.tile([1, HIDDEN], F32, name="o_row", tag="outrow")
        for n in range(8):
            nc.tensor.matmul(
                chunks1[n][:], eighth[:], x_row[:, n * 512:(n + 1) * 512],
                start=False, stop=True,
            )
            nc.scalar.copy(o_row[:, n * 512:(n + 1) * 512], chunks1[n][:])

        ar1_in = dram.tile([HIDDEN], F32, name="ar1_in")
        ar1_out = dram.tile([HIDDEN], F32, name="ar1_out")
        nc.sync.dma_start(ar1_in[:], o_row[:])
        nc.gpsimd.collective_compute(
            "AllReduce", ALU.add,
            replica_groups=[list(range(N_CORES))],
            ins=[ar1_in[:].opt()], outs=[ar1_out[:].opt()],
        )

        # ---- MLP ----
        h2_cols = rmsnorm_cols(ar1_out[:], i["ffn_norm"], "b")
        x2_row = sm.tile([1, HIDDEN], F32, name="x2_row", tag="xrow")
        nc.sync.dma_start(x2_row[:], ar1_out[:].rearrange("(a d) -> a d", a=1))

        # two tiles (separate banks): start/stop must cover a consistent
        # partition count per zero region, and the 96-row tail block differs.
        f1a = pp.tile([128, 10], F32, name="f1a", tag="ps")
        f1b = pp.tile([96, 1], F32, name="f1b", tag="ps")
        for t in range(16):
            wt = wpool.tile([128, 2, FF_N], F32, name="wf1_t", tag="w")
            nc.sync.dma_start(
                wt[:],
                i["wf1"][t * 256:(t + 1) * 256, :].rearrange(
                    "(b p) c -> p b c", p=128),
            )
            for half in range(2):
                kb = 2 * t + half
                for mb in range(11):
                    sz = FF_KB_SIZES[mb]
                    out = f1a[:, mb:mb + 1] if mb < 10 else f1b[:]
                    nc.tensor.matmul(
                        out,
                        wt[:, half, mb * 128:mb * 128 + sz],
                        h2_cols[:, kb:kb + 1],
                        start=(kb == 0 and mb in (0, 10)),
                        stop=(kb == KB - 1 and mb in (9, 10)),
                    )
        a_sb = sm.tile([128, 11], F32, name="a_sb")
        sig = sm.tile([128, 11], F32, name="sig")
        # silu(x) = x * sigmoid(x)
        nc.scalar.activation(sig[0:96, 10:11], f1b[:], AF.Sigmoid)
        nc.scalar.activation(sig[:, 0:10], f1a[:], AF.Sigmoid)
        nc.vector.tensor_tensor(a_sb[0:96, 10:11], f1b[:],
                                sig[0:96, 10:11], ALU.mult)
        nc.vector.tensor_tensor(a_sb[:, 0:10], f1a[:],
                                sig[:, 0:10], ALU.mult)

        chunks2 = [pp.tile([1, 512], F32, name=f"c2_{n}", tag="ps")
                   for n in range(8)]
        for kb in range(11):
            sz = FF_KB_SIZES[kb]
            wt = wpool.tile([sz, HIDDEN], F32, name="wf2_t", tag="w")
            nc.sync.dma_start(wt[:], i["wf2"][kb * 128:kb * 128 + sz, :])
            for n in range(8):
                nc.tensor.matmul(
                    chunks2[n][:], a_sb[0:sz, kb:kb + 1],
                    wt[:, n * 512:(n + 1) * 512],
                    start=(kb == 0), stop=False,
                )
        ff_row = sm.tile([1, HIDDEN], F32, name="ff_row", tag="outrow")
        for n in range(8):
            nc.tensor.matmul(
                chunks2[n][:], eighth[:], x2_row[:, n * 512:(n + 1) * 512],
                start=False, stop=True,
            )
            nc.scalar.copy(ff_row[:, n * 512:(n + 1) * 512], chunks2[n][:])

        ar2_in = dram.tile([HIDDEN], F32, name="ar2_in")
        ar2_out = dram.tile([HIDDEN], F32, name="ar2_out")
        nc.sync.dma_start(ar2_in[:], ff_row[:])
        nc.gpsimd.collective_compute(
            "AllReduce", ALU.add,
            replica_groups=[list(range(N_CORES))],
            ins=[ar2_in[:].opt()], outs=[ar2_out[:].opt()],
        )
        nc.sync.dma_start(y[:], ar2_out[:])


_BUILT = None


def _build():
    global _BUILT
    if _BUILT is None:
        nc = bacc.Bacc("TRN2", target_bir_lowering=False, debug=False,
                       num_devices=N_CORES)
        with tile.TileContext(nc) as tc:
            _emit(nc, tc)
        nc.compile()
        _BUILT = nc
    return _BUILT


def _shard(inputs):
    f = lambda a: np.ascontiguousarray(np.asarray(a, dtype=np.float32))
    x = f(inputs["x"])
    attn_norm = f(inputs["attn_norm"])
    ffn_norm = f(inputs["ffn_norm"])
    pos = int(np.asarray(inputs["pos"]))
    sin = f(inputs["sin_cache"][pos])
    cos = f(inputs["cos_cache"][pos])
    wq, wk, wv = f(inputs["w_q"]), f(inputs["w_k"]), f(inputs["w_v"])
    wo, wf1, wf2 = f(inputs["w_o"]), f(inputs["w_ff1"]), f(inputs["w_ff2"])
    kc = f(inputs["k_cache"]).reshape(KV_LEN, N_HEADS * HEAD_DIM)
    vc = f(inputs["v_cache"]).reshape(KV_LEN, N_HEADS * HEAD_DIM)

    in_maps = []
    for c in range(N_CORES):
        qs = slice(c * QKV_N, (c + 1) * QKV_N)
        fs = slice(c * FF_N, (c + 1) * FF_N)
        in_maps.append({
            "x": x,
            "ident32": np.eye(32, dtype=np.float32),
            "attn_norm": attn_norm,
            "ffn_norm": ffn_norm,
            "sin": sin,
            "cos": cos,
            "wq": np.ascontiguousarray(wq[:, qs]),
            "wk": np.ascontiguousarray(wk[:, qs]),
            "wv": np.ascontiguousarray(wv[:, qs]),
            "wo": np.ascontiguousarray(wo[qs, :]),
            "kc": np.ascontiguousarray(kc[:, qs]),
            "vc": np.ascontiguousarray(vc[:, qs]),
            "wf1": np.ascontiguousarray(wf1[:, fs]),
            "wf2": np.ascontiguousarray(wf2[fs, :]),
        })
    return in_maps


def kernel(**inputs):
    nc = _build()
    in_maps = _shard(inputs)
    res = bass_utils.run_bass_kernel_spmd(
        nc, in_maps, core_ids=list(range(N_CORES)))
    return res.results[0]["y"]



# revision 32
# speedup vs baseline: 1.6995x; 1.6995x over previous
"""Llama decode block (single token) on 8 TRN2 NeuronCores, tensor-parallel.

Sharding (per sharding_hint): w_q/w_k/w_v/w_ff1 column-sharded, w_o/w_ff2
row-sharded, KV cache sharded by head (4 heads/core). AllReduce after the
attention output projection and after w_ff2. Residuals (x, x2) are added
locally after each AllReduce, so the AR payloads carry only matvec partials.

Memory-bound problem -> minimize HBM bytes and DMA descriptor count:

* Every weight is split on the host into a float16 "hi" part plus a
  float8e4 "lo" part holding 2^12*(W - hi).  3 bytes/elem instead of 4,
  with ~1e-5 effective relative error (fp16 residual quantized by fp8).
* All tensors are pre-packed on the host into the exact SBUF tile layout
  ([128 partitions, r, cols], row g(p,r)=r*128+p), so every DMA moves
  fully contiguous 16-32KB per-partition lines (the f32 baseline was
  descriptor-rate-bound at ~181 GB/s on 2KB strided descriptors).
* KV cache is fp16 (fp16 scores/attn keeps max rel err ~6e-3 vs the 2e-2
  gate; bf16 fails it).

Matvec scheme (per k-block): activation h is kept as an fp16 dual
(h1=fp16(h), h2=fp16(h-h1)) -> matmul with 2 stationary columns gives
psum rows [h1@Whi ; h2@Whi] at no extra PE cost; the lo pass uses
hs=fp16(h*2^-12) against Wlo=fp8(2^12*residual) and accumulates onto row
0, so products come out correctly scaled with no fixup pass. Row pairs
are combined either by the next matmul's contraction (q replicate, v
new-token), by PE transposes (ff1 -> silu columns), or by a SWDGE
accumulate DMA when writing the AllReduce input (wo, ff2).
"""

import math

import numpy as np
import ml_dtypes

import concourse.bass as bass
import concourse.mybir as mybir
import concourse.tile as tile
from concourse import bacc
from concourse import bass_utils

F32 = mybir.dt.float32
F16 = mybir.dt.float16
FP8 = mybir.dt.float8e4
AF = mybir.ActivationFunctionType
ALU = mybir.AluOpType

HIDDEN = 4096
N_HEADS = 32
HEAD_DIM = 128
INTERM = 11008
KV_LEN = 4096
N_CORES = 8

HEADS_PC = N_HEADS // N_CORES          # 4 heads per core
QKV_N = HEADS_PC * HEAD_DIM            # 512
FF_N = INTERM // N_CORES               # 1376
FF_NP = 1408                           # padded to 11*128
KB = HIDDEN // 128                     # 32 k-blocks of the hidden dim
SCALE = 1.0 / math.sqrt(HEAD_DIM)
LO = 2.0 ** 12                         # residual scale for the fp8 stream
ILO = 1.0 / LO

NP_FP8 = ml_dtypes.float8_e4m3

DEBUG = False          # emit intermediate tensors as extra outputs

# weight-stream tiling (r-blocks per DMA tile)
QKV_RT = [16, 16]                      # 32 kb in 2 tiles of [128,16,512]
WO_RT = [2, 2]                         # 4 r in 2 tiles of [128,2,4096]
FF1_RT = [6, 6, 6, 6, 6, 2]            # 32 kb over [128,r,1408] tiles
FF2_RT = [2, 2, 2, 2, 2, 1]            # 11 kb over [128,r,4096] tiles
FF1_CH = [(0, 512), (512, 512), (1024, 384)]   # ff1 psum column chunks


def _emit(nc, tc):
    i = {}

    def din(name, shape, dt=F32):
        i[name] = nc.dram_tensor(name, list(shape), dt, kind="ExternalInput").ap()

    din("x", [HIDDEN])
    din("attn_norm", [HIDDEN])
    din("ffn_norm", [HIDDEN])
    din("sinq", [2, 64])               # pre-scaled by 1/sqrt(d)
    din("cosq", [2, 64])
    din("sink", [2, 64])
    din("cosk", [2, 64])
    din("ident32", [32, 32])
    for w in ("wq", "wk", "wv"):
        din(w + "_hi", [128, KB, QKV_N], F16)
        din(w + "_lo", [128, KB, QKV_N], FP8)
    din("wo_hi", [128, HEADS_PC, HIDDEN], F16)
    din("wo_lo", [128, HEADS_PC, HIDDEN], FP8)
    din("wf1_hi", [128, KB, FF_NP], F16)
    din("wf1_lo", [128, KB, FF_NP], FP8)
    din("wf2_hi", [128, 11, HIDDEN], F16)
    din("wf2_lo", [128, 11, HIDDEN], FP8)
    din("kc", [4, 128, 8, QKV_N], F16)
    din("vc", [4, 128, 8, QKV_N], F16)
    y = nc.dram_tensor("y", [HIDDEN], F32, kind="ExternalOutput").ap()

    dbg_outs = {}

    def dbg(name, src_ap, shape):
        if not DEBUG:
            return
        d = nc.dram_tensor("dbg_" + name, list(shape), F32,
                           kind="ExternalOutput").ap()
        nc.sync.dma_start(d, src_ap)
        dbg_outs[name] = d

    with (
        tc.tile_pool(name="const", bufs=1) as cpool,
        tc.tile_pool(name="whi", bufs=4) as hpool,
        tc.tile_pool(name="wlo", bufs=4) as lpool,
        tc.tile_pool(name="kpool", bufs=2) as kpool,
        tc.tile_pool(name="vpool", bufs=2) as vpool,
        tc.tile_pool(name="sm", bufs=1) as sm,
        tc.tile_pool(name="scr", bufs=2) as scr,
        tc.tile_pool(name="psum", bufs=8, space="PSUM") as pp,
        tc.tile_pool(name="dram", bufs=1, space="DRAM") as dram,
    ):
        # ---- constants ----
        ones32 = cpool.tile([32, 1], F32)
        nc.vector.memset(ones32[:], 1.0)
        ones128 = cpool.tile([128, 1], F32)
        nc.vector.memset(ones128[:], 1.0)
        ones_r32 = cpool.tile([1, 32], F32)
        nc.vector.memset(ones_r32[:], 1.0)
        ones_r128 = cpool.tile([1, 128], F32)
        nc.vector.memset(ones_r128[:], 1.0)
        ones2_128 = cpool.tile([2, 128], F32)
        nc.vector.memset(ones2_128[:], 1.0)
        ones2_1 = cpool.tile([2, 1], F32)
        nc.vector.memset(ones2_1[:], 1.0)
        ones1_2 = cpool.tile([1, 2], F32)
        nc.vector.memset(ones1_2[:], 1.0)
        eps11 = cpool.tile([1, 1], F32)
        nc.vector.memset(eps11[:], 1e-6)
        ident32 = cpool.tile([32, 32], F32)
        nc.sync.dma_start(ident32[:], i["ident32"])
        trig = {}
        for t in ("sinq", "cosq", "sink", "cosk"):
            trig[t] = cpool.tile([2, 64], F32, name=t)
            nc.sync.dma_start(trig[t][:], i[t])

        x_rows = cpool.tile([32, 128], F32)
        nc.sync.dma_start(x_rows[:], i["x"].rearrange("(a d) -> a d", a=32))
        anorm_rows = cpool.tile([32, 128], F32)
        nc.sync.dma_start(anorm_rows[:],
                          i["attn_norm"].rearrange("(a d) -> a d", a=32))
        fnorm_rows = cpool.tile([32, 128], F32)
        nc.sync.dma_start(fnorm_rows[:],
                          i["ffn_norm"].rearrange("(a d) -> a d", a=32))

        # ---- rmsnorm -> fp16 dual columns hd[128,32,2], hs[128,32] ----
        def rmsnorm_dual(xr, nr, tag):
            sq = sm.tile([32, 128], F32, name=f"sq_{tag}")
            ssq = sm.tile([32, 1], F32, name=f"ssq_{tag}")
            nc.scalar.activation(sq[:], xr[:], AF.Square, accum_out=ssq[:])
            ms_ps = pp.tile([1, 1], F32, name=f"ms_{tag}", tag="ps")
            nc.tensor.matmul(ms_ps[:], ones32[:], ssq[:])
            rstd = sm.tile([1, 1], F32, name=f"rstd_{tag}")
            nc.scalar.activation(rstd[:], ms_ps[:], AF.Sqrt,
                                 bias=eps11[:], scale=1.0 / HIDDEN)
            nc.vector.reciprocal(rstd[:], rstd[:])
            rstd_ps = pp.tile([32, 1], F32, name=f"rstdp_{tag}", tag="ps")
            nc.tensor.matmul(rstd_ps[:], ones_r32[:], rstd[:])
            rstd32 = sm.tile([32, 1], F32, name=f"rstd32_{tag}")
            nc.vector.tensor_copy(rstd32[:], rstd_ps[:])
            h_rows = sm.tile([32, 128], F32, name=f"hr_{tag}")
            nc.vector.tensor_tensor(h_rows[:], xr[:], nr[:], ALU.mult)
            nc.vector.tensor_scalar_mul(h_rows[:], h_rows[:], rstd32[:])
            h_ps = pp.tile([128, 32], F32, name=f"hps_{tag}", tag="ps")
            nc.tensor.transpose(h_ps[:], h_rows[:], ident32[:])
            h_cols = sm.tile([128, 32], F32, name=f"hc_{tag}")
            nc.vector.tensor_copy(h_cols[:], h_ps[:])
            hd = sm.tile([128, 32, 2], F16, name=f"hd_{tag}")
            nc.vector.tensor_copy(hd[:, :, 0], h_cols[:])
            tmp = sm.tile([128, 32], F32, name=f"htmp_{tag}")
            nc.vector.tensor_tensor(tmp[:], h_cols[:], hd[:, :, 0], ALU.subtract)
            nc.vector.tensor_copy(hd[:, :, 1], tmp[:])
            hs = sm.tile([128, 32], F16, name=f"hs_{tag}")
            nc.vector.tensor_scalar_mul(hs[:], h_cols[:], ILO)
            return hd, hs

        hd, hs = rmsnorm_dual(x_rows, anorm_rows, "a")

        # ---- q/k/v: one psum bank per projection (one accumulation group
        # per bank); lo accumulates on row 0, dual correction on row 1 ----
        qkv_ps = {w: pp.tile([2, QKV_N], F32, name=f"{w}_ps", tag="ps")
                  for w in ("wq", "wk", "wv")}
        for wi, w in enumerate(("wq", "wk", "wv")):
            ps = qkv_ps[w]
            kb0 = 0
            for t, rt in enumerate(QKV_RT):
                hi_t = hpool.tile([128, rt, QKV_N], F16, name="qkv_hi", tag="whi")
                nc.sync.dma_start(hi_t[:], i[w + "_hi"][:, kb0:kb0 + rt, :])
                lo_t = lpool.tile([128, rt, QKV_N], FP8, name="qkv_lo", tag="wlo")
                nc.sync.dma_start(lo_t[:], i[w + "_lo"][:, kb0:kb0 + rt, :])
                for b in range(rt):
                    kb = kb0 + b
                    nc.tensor.matmul(
                        ps[0:2, :], hd[:, kb, :], hi_t[:, b, :],
                        start=(kb == 0), stop=False,
                    )
                    nc.tensor.matmul(
                        ps[0:1, :], hs[:, kb:kb + 1], lo_t[:, b, :],
                        start=False, stop=(kb == KB - 1),
                    )
                kb0 += rt

        q_sb = sm.tile([2, QKV_N], F32, name="q_sb")
        nc.vector.tensor_copy(q_sb[:], qkv_ps["wq"][:])
        k_sb = sm.tile([2, QKV_N], F32, name="k_sb")
        nc.vector.tensor_copy(k_sb[:], qkv_ps["wk"][:])
        v16 = sm.tile([2, QKV_N], F16, name="v16")
        nc.vector.tensor_copy(v16[:], qkv_ps["wv"][:])

        # ---- RoPE (dual rows; q uses trig pre-scaled by 1/sqrt(d)) ----
        def rope(src, sin_t, cos_t, tag):
            out = sm.tile([2, QKV_N], F32, name=f"rope_{tag}")
            tmp = sm.tile([2, QKV_N], F32, name=f"ropetmp_{tag}")
            r3 = src[:].rearrange("p (h d) -> p h d", h=HEADS_PC)
            o3 = out[:].rearrange("p (h d) -> p h d", h=HEADS_PC)
            t3 = tmp[:].rearrange("p (h d) -> p h d", h=HEADS_PC)
            cb = cos_t[:].unsqueeze(1).to_broadcast((2, HEADS_PC, 64))
            sb = sin_t[:].unsqueeze(1).to_broadcast((2, HEADS_PC, 64))
            x1, x2 = r3[:, :, 0:64], r3[:, :, 64:128]
            nc.vector.tensor_tensor(o3[:, :, 0:64], x1, cb, ALU.mult)
            nc.vector.tensor_tensor(t3[:, :, 0:64], x2, sb, ALU.mult)
            nc.vector.tensor_sub(o3[:, :, 0:64], o3[:, :, 0:64],
                                 t3[:, :, 0:64])
            nc.vector.tensor_tensor(o3[:, :, 64:128], x2, cb, ALU.mult)
            nc.vector.tensor_tensor(t3[:, :, 64:128], x1, sb, ALU.mult)
            nc.vector.tensor_add(o3[:, :, 64:128], o3[:, :, 64:128],
                                 t3[:, :, 64:128])
            return out

        dbg("q_sb", q_sb[:], [2, QKV_N])
        dbg("k_sb", k_sb[:], [2, QKV_N])

        rope_q = rope(q_sb, trig["sinq"], trig["cosq"], "q")
        rope_k = rope(k_sb, trig["sink"], trig["cosk"], "k")
        dbg("rope_q", rope_q[:], [2, QKV_N])

        # q replicated to 128 partitions; the ones-matmul also sums the dual
        qrep_ps = pp.tile([128, QKV_N], F32, name="qrep_ps", tag="ps")
        nc.tensor.matmul(qrep_ps[:], ones2_128[:], rope_q[:])
        q_rep = sm.tile([128, QKV_N], F32, name="q_rep")
        nc.vector.tensor_copy(q_rep[:], qrep_ps[:])

        # current-token score: combine k dual via ones-matmul, then q.k
        kc_ps = pp.tile([1, QKV_N], F32, name="kc_ps", tag="ps")
        nc.tensor.matmul(kc_ps[:], ones2_1[:], rope_k[:])
        k_comb = sm.tile([1, QKV_N], F32, name="k_comb")
        nc.vector.tensor_copy(k_comb[:], kc_ps[:])
        prod_new = sm.tile([1, QKV_N], F32, name="prod_new")
        nc.vector.tensor_tensor(prod_new[:], k_comb[:], q_rep[0:1, :],
                                ALU.mult)
        s_new = sm.tile([1, HEADS_PC], F32, name="s_new")
        nc.vector.tensor_reduce(
            s_new[:], prod_new[:].rearrange("p (h d) -> p h d", h=HEADS_PC),
            mybir.AxisListType.X, ALU.add)
        e_new = sm.tile([1, HEADS_PC], F32, name="e_new")
        nc.scalar.activation(e_new[:], s_new[:], AF.Exp)
        e2_ps = pp.tile([2, HEADS_PC], F32, name="e2_ps", tag="ps")
        nc.tensor.matmul(e2_ps[:], ones1_2[:], e_new[:])
        e_new2 = sm.tile([2, HEADS_PC], F16, name="e_new2")
        nc.vector.tensor_copy(e_new2[:], e2_ps[:])

        # ---- attention over the KV cache ----
        o_ps = pp.tile([128, HEADS_PC], F32, name="o_ps", tag="ps")
        denom_acc = sm.tile([128, HEADS_PC], F32, name="denom_acc")
        nc.vector.memset(denom_acc[:], 0.0)

        for st in range(4):
            k_sup = kpool.tile([128, 8, QKV_N], F16, name="k_sup", tag="k")
            v_sup = vpool.tile([128, 8, QKV_N], F16, name="v_sup", tag="v")
            nc.sync.dma_start(k_sup[:], i["kc"][st])
            nc.sync.dma_start(v_sup[:], i["vc"][st])
            for b in range(8):
                prod = scr.tile([128, QKV_N], F16, name="prod", tag="prod")
                nc.vector.tensor_tensor(prod[:], k_sup[:, b, :], q_rep[:],
                                        ALU.mult)
                scores = scr.tile([128, HEADS_PC], F32, name="scores", tag="sc")
                nc.vector.tensor_reduce(
                    scores[:],
                    prod[:].rearrange("p (h d) -> p h d", h=HEADS_PC),
                    mybir.AxisListType.X, ALU.add)
                expt = scr.tile([128, HEADS_PC], F32, name="expt", tag="ex")
                nc.scalar.activation(expt[:], scores[:], AF.Exp)
                nc.vector.tensor_add(denom_acc[:], denom_acc[:], expt[:])
                expt16 = scr.tile([128, HEADS_PC], F16, name="expt16", tag="e16")
                nc.scalar.copy(expt16[:], expt[:])
                for h in range(HEADS_PC):
                    nc.tensor.matmul(
                        o_ps[:, h:h + 1],
                        v_sup[:, b, h * 128:(h + 1) * 128],
                        expt16[:, h:h + 1],
                        start=(st == 0 and b == 0 and h == 0), stop=False,
                    )
        for h in range(HEADS_PC):
            nc.tensor.matmul(
                o_ps[:, h:h + 1], v16[:, h * 128:(h + 1) * 128],
                e_new2[:, h:h + 1],
                start=False, stop=(h == HEADS_PC - 1),
            )

        # denom = sum over tokens of the same fp16 expt + e_new
        d_ps = pp.tile([1, HEADS_PC], F32, name="d_ps", tag="ps")
        nc.tensor.matmul(d_ps[:], ones128[:], denom_acc[:])
        denom = sm.tile([1, HEADS_PC], F32, name="denom")
        nc.vector.tensor_copy(denom[:], d_ps[:])
        nc.vector.tensor_add(denom[:], denom[:], e_new[:])
        nc.vector.reciprocal(denom[:], denom[:])
        r_ps = pp.tile([128, HEADS_PC], F32, name="r_ps", tag="ps")
        nc.tensor.matmul(r_ps[:], ones_r128[:], denom[:])
        recip_bc = sm.tile([128, HEADS_PC], F32, name="recip_bc")
        nc.vector.tensor_copy(recip_bc[:], r_ps[:])
        o_sb = sm.tile([128, HEADS_PC], F32, name="o_sb")
        nc.vector.tensor_tensor(o_sb[:], o_ps[:], recip_bc[:], ALU.mult)
        dbg("denom", denom[:], [1, HEADS_PC])
        dbg("o_sb", o_sb[:], [128, HEADS_PC])

        od = sm.tile([128, HEADS_PC, 2], F16, name="od")
        nc.vector.tensor_copy(od[:, :, 0], o_sb[:])
        otmp = sm.tile([128, HEADS_PC], F32, name="otmp")
        nc.vector.tensor_tensor(otmp[:], o_sb[:], od[:, :, 0], ALU.subtract)
        nc.vector.tensor_copy(od[:, :, 1], otmp[:])
        os_ = sm.tile([128, HEADS_PC], F16, name="os_")
        nc.vector.tensor_scalar_mul(os_[:], o_sb[:], ILO)

        # ---- o @ w_o: 8 output chunks, one psum bank each ----
        wo_ps = [pp.tile([2, 512], F32, name=f"wo_ps{n}", tag="ps")
                 for n in range(8)]
        r0 = 0
        for t, rt in enumerate(WO_RT):
            hi_t = hpool.tile([128, rt, HIDDEN], F16, name="wo_hi", tag="whi")
            nc.sync.dma_start(hi_t[:], i["wo_hi"][:, r0:r0 + rt, :])
            lo_t = lpool.tile([128, rt, HIDDEN], FP8, name="wo_lo", tag="wlo")
            nc.sync.dma_start(lo_t[:], i["wo_lo"][:, r0:r0 + rt, :])
            for b in range(rt):
                r = r0 + b
                for n in range(8):
                    nc.tensor.matmul(
                        wo_ps[n][0:2, :], od[:, r, :],
                        hi_t[:, b, 512 * n:512 * n + 512],
                        start=(r == 0), stop=False,
                    )
                    nc.tensor.matmul(
                        wo_ps[n][0:1, :], os_[:, r:r + 1],
                        lo_t[:, b, 512 * n:512 * n + 512],
                        start=False, stop=(r == HEADS_PC - 1),
                    )
            r0 += rt
        # stage as [2, 4096]: row 0 = hi+lo part, row 1 = dual correction;
        # the SWDGE pair below writes row 0 then accumulates row 1 on DRAM
        wo_sb = sm.tile([2, HIDDEN], F32, name="wo_sb")
        for n in range(8):
            nc.vector.tensor_copy(wo_sb[0:2, 512 * n:512 * n + 512],
                                  wo_ps[n][:])

        dbg("wo_sb", wo_sb[:], [2, HIDDEN])
        ar1_in = dram.tile([HIDDEN], F32, name="ar1_in")
        ar1_out = dram.tile([HIDDEN], F32, name="ar1_out")
        ar1v = ar1_in[:].rearrange("(a d) -> a d", a=1)
        nc.gpsimd.dma_start(ar1v, wo_sb[0:1, :])
        nc.gpsimd.dma_start(ar1v, wo_sb[1:2, :], accum_op=ALU.add)
        nc.gpsimd.collective_compute(
            "AllReduce", ALU.add,
            replica_groups=[list(range(N_CORES))],
            ins=[ar1_in[:].opt()], outs=[ar1_out[:].opt()],
        )

        # ---- MLP ----
        ar1_rows = sm.tile([32, 128], F32, name="ar1_rows")
        nc.sync.dma_start(ar1_rows[:], ar1_out[:].rearrange("(a d) -> a d", a=32))
        x2_rows = sm.tile([32, 128], F32, name="x2_rows")
        nc.vector.tensor_add(x2_rows[:], x_rows[:], ar1_rows[:])
        dbg("x2_rows", x2_rows[:], [32, 128])

        hd2, hs2 = rmsnorm_dual(x2_rows, fnorm_rows, "b")

        f1_ps = [pp.tile([2, 512], F32, name=f"f1_ps{n}", tag="ps")
                 for n in range(3)]
        kb0 = 0
        for t, rt in enumerate(FF1_RT):
            hi_t = hpool.tile([128, 6, FF_NP], F16, name="f1_hi", tag="whi")
            lo_t = lpool.tile([128, 6, FF_NP], FP8, name="f1_lo", tag="wlo")
            nc.sync.dma_start(hi_t[:, 0:rt, :], i["wf1_hi"][:, kb0:kb0 + rt, :])
            nc.sync.dma_start(lo_t[:, 0:rt, :], i["wf1_lo"][:, kb0:kb0 + rt, :])
            for b in range(rt):
                kb = kb0 + b
                for n, (c0, w) in enumerate(FF1_CH):
                    nc.tensor.matmul(
                        f1_ps[n][0:2, 0:w], hd2[:, kb, :],
                        hi_t[:, b, c0:c0 + w],
                        start=(kb == 0), stop=False,
                    )
                    nc.tensor.matmul(
                        f1_ps[n][0:1, 0:w], hs2[:, kb:kb + 1],
                        lo_t[:, b, c0:c0 + w],
                        start=False, stop=(kb == KB - 1),
                    )
            kb0 += rt
        f1_sb = [sm.tile([2, 512], F32, name=f"f1_sb{n}") for n in range(3)]
        for n in range(3):
            nc.vector.tensor_copy(f1_sb[n][:], f1_ps[n][:])

        # a-columns via PE transposes of each dual pair (128-col windows)
        acol_ps = pp.tile([128, 22], F32, name="acol_ps", tag="ps")
        ident2 = ident32[0:2, 0:2]
        for j in range(11):
            n = (128 * j) // 512
            off = 128 * j - 512 * n
            nc.tensor.transpose(acol_ps[:, 2 * j:2 * j + 2],
                                f1_sb[n][:, off:off + 128], ident2)
        acol_sb = sm.tile([128, 22], F32, name="acol_sb")
        nc.vector.tensor_copy(acol_sb[:], acol_ps[:])
        pre = sm.tile([128, 11], F32, name="pre_silu")
        a3 = acol_sb[:].rearrange("p (j t) -> p j t", t=2)
        nc.vector.tensor_tensor(pre[:], a3[:, :, 0], a3[:, :, 1], ALU.add)
        sig = sm.tile([128, 11], F32, name="sig")
        nc.scalar.activation(sig[:], pre[:], AF.Sigmoid)
        a_sb = sm.tile([128, 11], F32, name="a_sb")
        nc.vector.tensor_tensor(a_sb[:], pre[:], sig[:], ALU.mult)
        dbg("a_sb", a_sb[:], [128, 11])
        ad = sm.tile([128, 11, 2], F16, name="ad")
        nc.vector.tensor_copy(ad[:, :, 0], a_sb[:])
        atmp = sm.tile([128, 11], F32, name="atmp")
        nc.vector.tensor_tensor(atmp[:], a_sb[:], ad[:, :, 0], ALU.subtract)
        nc.vector.tensor_copy(ad[:, :, 1], atmp[:])
        as_ = sm.tile([128, 11], F16, name="as_")
        nc.vector.tensor_scalar_mul(as_[:], a_sb[:], ILO)

        f2_ps = [pp.tile([2, 512], F32, name=f"f2_ps{n}", tag="ps")
                 for n in range(8)]
        kb0 = 0
        for t, rt in enumerate(FF2_RT):
            hi_t = hpool.tile([128, rt, HIDDEN], F16, name="f2_hi", tag="whi")
            nc.sync.dma_start(hi_t[:], i["wf2_hi"][:, kb0:kb0 + rt, :])
            lo_t = lpool.tile([128, rt, HIDDEN], FP8, name="f2_lo", tag="wlo")
            nc.sync.dma_start(lo_t[:], i["wf2_lo"][:, kb0:kb0 + rt, :])
            for b in range(rt):
                kb = kb0 + b
                for n in range(8):
                    nc.tensor.matmul(
                        f2_ps[n][0:2, :], ad[:, kb, :],
                        hi_t[:, b, 512 * n:512 * n + 512],
                        start=(kb == 0), stop=False,
                    )
                    nc.tensor.matmul(
                        f2_ps[n][0:1, :], as_[:, kb:kb + 1],
                        lo_t[:, b, 512 * n:512 * n + 512],
                        start=False, stop=(kb == 10),
                    )
            kb0 += rt
        ff_sb = sm.tile([2, HIDDEN], F32, name="ff_sb")
        for n in range(8):
            nc.vector.tensor_copy(ff_sb[0:2, 512 * n:512 * n + 512],
                                  f2_ps[n][:])

        ar2_in = dram.tile([HIDDEN], F32, name="ar2_in")
        ar2_out = dram.tile([HIDDEN], F32, name="ar2_out")
        ar2v = ar2_in[:].rearrange("(a d) -> a d", a=1)
        nc.gpsimd.dma_start(ar2v, ff_sb[0:1, :])
        nc.gpsimd.dma_start(ar2v, ff_sb[1:2, :], accum_op=ALU.add)
        nc.gpsimd.collective_compute(
            "AllReduce", ALU.add,
            replica_groups=[list(range(N_CORES))],
            ins=[ar2_in[:].opt()], outs=[ar2_out[:].opt()],
        )

        ar2_rows = sm.tile([32, 128], F32, name="ar2_rows")
        nc.sync.dma_start(ar2_rows[:], ar2_out[:].rearrange("(a d) -> a d", a=32))
        y_rows = sm.tile([32, 128], F32, name="y_rows")
        nc.vector.tensor_add(y_rows[:], x2_rows[:], ar2_rows[:])
        nc.sync.dma_start(y.rearrange("(a d) -> a d", a=32), y_rows[:])


_BUILT = None


def _build():
    global _BUILT
    if _BUILT is None:
        nc = bacc.Bacc("TRN2", target_bir_lowering=False, debug=False,
                       num_devices=N_CORES)
        with tile.TileContext(nc) as tc:
            _emit(nc, tc)
        nc.compile()
        _BUILT = nc
    return _BUILT


def _hilo(W):
    hi = W.astype(np.float16)
    res = (W - hi.astype(np.float32)) * LO
    lo = np.clip(res, -224.0, 224.0).astype(NP_FP8)
    return hi, lo


def _pack_rc(A, r128):
    """[r128*128, C] -> [128, r128, C] with row r*128+p on partition p."""
    C = A.shape[1]
    return np.ascontiguousarray(A.reshape(r128, 128, C).transpose(1, 0, 2))


def _shard(inputs):
    f = lambda a: np.ascontiguousarray(np.asarray(a, dtype=np.float32))
    x = f(inputs["x"])
    attn_norm = f(inputs["attn_norm"])
    ffn_norm = f(inputs["ffn_norm"])
    pos = int(np.asarray(inputs["pos"]))
    sin = f(inputs["sin_cache"][pos])
    cos = f(inputs["cos_cache"][pos])
    sinq = np.ascontiguousarray(np.stack([sin * SCALE] * 2).astype(np.float32))
    cosq = np.ascontiguousarray(np.stack([cos * SCALE] * 2).astype(np.float32))
    sink = np.ascontiguousarray(np.stack([sin] * 2).astype(np.float32))
    cosk = np.ascontiguousarray(np.stack([cos] * 2).astype(np.float32))
    wq, wk, wv = f(inputs["w_q"]), f(inputs["w_k"]), f(inputs["w_v"])
    wo, wf1, wf2 = f(inputs["w_o"]), f(inputs["w_ff1"]), f(inputs["w_ff2"])
    kc = f(inputs["k_cache"]).reshape(KV_LEN, N_HEADS * HEAD_DIM)
    vc = f(inputs["v_cache"]).reshape(KV_LEN, N_HEADS * HEAD_DIM)
    ident32 = np.eye(32, dtype=np.float32)

    in_maps = []
    for c in range(N_CORES):
        qs = slice(c * QKV_N, (c + 1) * QKV_N)
        fs = slice(c * FF_N, (c + 1) * FF_N)
        m = {
            "x": x, "attn_norm": attn_norm, "ffn_norm": ffn_norm,
            "sinq": sinq, "cosq": cosq, "sink": sink, "cosk": cosk,
            "ident32": ident32,
        }
        for name, wfull in (("wq", wq), ("wk", wk), ("wv", wv)):
            hi, lo = _hilo(wfull[:, qs])
            m[name + "_hi"] = _pack_rc(hi, KB)
            m[name + "_lo"] = _pack_rc(lo, KB)
        hi, lo = _hilo(wo[qs, :])
        m["wo_hi"] = _pack_rc(hi, HEADS_PC)
        m["wo_lo"] = _pack_rc(lo, HEADS_PC)
        w1 = np.pad(wf1[:, fs], ((0, 0), (0, FF_NP - FF_N)))
        hi, lo = _hilo(w1)
        m["wf1_hi"] = _pack_rc(hi, KB)
        m["wf1_lo"] = _pack_rc(lo, KB)
        w2 = np.pad(wf2[fs, :], ((0, FF_NP - FF_N), (0, 0)))
        hi, lo = _hilo(w2)
        m["wf2_hi"] = _pack_rc(hi, 11)
        m["wf2_lo"] = _pack_rc(lo, 11)
        # KV: [4096, 512] -> [4 supertiles, 128 part(=token%128), 8, 512]
        m["kc"] = np.ascontiguousarray(
            kc[:, qs].astype(np.float16).reshape(4, 8, 128, QKV_N)
            .transpose(0, 2, 1, 3))
        m["vc"] = np.ascontiguousarray(
            vc[:, qs].astype(np.float16).reshape(4, 8, 128, QKV_N)
            .transpose(0, 2, 1, 3))
        in_maps.append(m)
    return in_maps


def kernel(**inputs):
    nc = _build()
    in_maps = _shard(inputs)
    res = bass_utils.run_bass_kernel_spmd(
        nc, in_maps, core_ids=list(range(N_CORES)))
    return res.results[0]["y"]
